# revision 82
# baseline (speedup 1.0000x reference)
"""Trainium2 Bass kernel v3: contrastive KL loss via moment expansion.

Per batch b (anchors p, targets t, T=2048):
    tw = pos/prs, pos = exp(-d2), d2 = (z_p - z_t)^2, z = tic_n/sqrt(2) - ZC
    iw = sc/rsum, sc = G+1 (cosine sim + 1), rsum = rs + T
    row_kl = ln(rs+T) - ln(prs) - (A + C)/prs
      prs = sum_t pos,  A = sum_t pos*d2,  C = sum_t pos*lnf,
      lnf = ln(G + bbar),  bbar = 1 + eps*(T + mean_p rs)   [symmetric bias]
      mean_p rs = |svec|^2 / T with svec = sum_t ne_t  (exact identity)

Moment restructure (validated ~1e-3 vs reference, tol 2e-2):
    pos ~ P(d2) deg-3 poly  ->  pos = sum_{i,j} Gam[i,j] z_p^i z_t^j  (rank 9)
    pos*d2 ~ H(d2) deg-4 poly (GamH)
    M[m,p] = sum_t z_t^m lnf[p,t] = sum_j V_j^T @ lnf_j   (PE, uses lnf symmetry)
    S[m]   = sum_t z_t^m
    Wt_blk = M_blk^T @ GamP^T  (PE, 9-col matmuls; fuses W matmul + transpose)
    AC[p]  = sum_i z_p^i Wt[p,i]   (DVE dot w/ accum)
    prs[p] = sum_i z_p^i (GamP @ S)_i,  A2[p] = sum_i z_p^i (GamH @ S)_i

v3 scheduling (vs v2 baseline at 315us):
    - 8 wide DMAs across 4 engine queues instead of 64 serialized on Sync
    - n2 squares on DVE (not ACT); dummy Sqrt at t=0 preloads sqrt table
    - bbar from |svec|^2: first Ln of each batch has no rs dependency
    - rs matvecs after each j-loop; prs/a2 dots + batch tails + next-batch
      Phase E interleaved into the j-loop so ACT (the ~147us Ln floor) never
      stalls and PE stays HAM-warm.
"""

import os
import numpy as np

os.environ.setdefault("MYCRO_LOCAL_CACHE", "1")

B, T, D = 32, 2048, 64
N_CORES = 8
B_LOC = B // N_CORES
NBLK = T // 128
EPS = 1e-6
ZC = 0.35
INV_SQRT2 = 0.7071067811865476
NP_DEG, NH_DEG = 3, 4
K = 2 * max(NP_DEG, NH_DEG) + 1      # 9 monomials z^0..z^8
NCOL = B_LOC * NBLK                  # 64 accumulator columns per core

_PROGRAM = None


def _fit_gammas():
    from math import comb
    xs = np.linspace(0.0, 0.5, 4001)
    cP = np.polyfit(xs, np.exp(-xs), NP_DEG)[::-1]
    cH = np.polyfit(xs, xs * np.exp(-xs), NH_DEG)[::-1]

    def gamma_of(c):
        g = np.zeros((K, K))
        for k_, ck in enumerate(c):
            for j in range(2 * k_ + 1):
                g[2 * k_ - j, j] += ck * comb(2 * k_, j) * (-1) ** j
        return g

    gp, gh = gamma_of(cP), gamma_of(cH)
    # device layout: gams[j, i] = Gam[i, j]  (lhsT for W = Gam @ M)
    return np.concatenate([gp.T, gh.T], axis=1).astype(np.float32)  # [K, 2K]


GAMS = _fit_gammas()


def _build_program():
    from contextlib import ExitStack
    import concourse.bass as bass
    import concourse.tile as tile
    from concourse import bacc, mybir, masks

    f32 = mybir.dt.float32
    bf16 = mybir.dt.bfloat16
    AF = mybir.ActivationFunctionType
    OP = mybir.AluOpType
    AX = mybir.AxisListType

    nc = bacc.Bacc(
        "TRN2", target_bir_lowering=False, debug=False, num_devices=N_CORES
    )
    emb_d = nc.dram_tensor("embedding", [B_LOC, T, D], f32, kind="ExternalInput").ap()
    tic_d = nc.dram_tensor("tic", [B_LOC, T], f32, kind="ExternalInput").ap()
    gam_d = nc.dram_tensor("gams", [K, 2 * K], f32, kind="ExternalInput").ap()
    out_d = nc.dram_tensor("out", [128, 1], f32, kind="ExternalOutput").ap()

    with tile.TileContext(nc) as tc, ExitStack() as ctx:
        consts = ctx.enter_context(tc.tile_pool(name="consts", bufs=1))
        accp = ctx.enter_context(tc.tile_pool(name="acc", bufs=1))
        small = ctx.enter_context(tc.tile_pool(name="small", bufs=12))
        psq = ctx.enter_context(tc.tile_pool(name="psq", bufs=3))
        pne = ctx.enter_context(tc.tile_pool(name="pne", bufs=18))
        plnf = ctx.enter_context(tc.tile_pool(name="plnf", bufs=6))
        pmw = ctx.enter_context(tc.tile_pool(name="pmw", bufs=4))
        # PSUM: pG 2x[128,1024]f32 = 4 banks, pM 1x[9,2048]f32 = 4 banks
        pG = ctx.enter_context(
            tc.tile_pool(name="pG", bufs=2, space=bass.MemorySpace.PSUM)
        )
        pM = ctx.enter_context(
            tc.tile_pool(name="pM", bufs=1, space=bass.MemorySpace.PSUM)
        )

        # ---- constants ----
        identity = consts.tile([128, 128], bf16)
        masks.make_identity(nc, identity[:])
        ones_row_f = consts.tile([1, 128], f32)
        nc.gpsimd.memset(ones_row_f[:], 1.0)
        ones_col_b = consts.tile([128, 1], bf16)
        nc.gpsimd.memset(ones_col_b[:], 1.0)
        gam_f = consts.tile([K, 2 * K], f32)
        nc.sync.dma_start(gam_f[:], gam_d)
        gam_b = consts.tile([K, K], bf16)
        nc.vector.tensor_copy(gam_b[:], gam_f[:, 0:K])
        bias_T = consts.tile([128, 1], f32)
        nc.gpsimd.memset(bias_T[:], float(T))
        # bbar = 1 + eps*(T + mean_p rs); mean_p rs ~ 1 +- 0.5 and eps=1e-6,
        # so a constant bias is accurate to ~1e-6 absolute in lnf
        bbar_c = consts.tile([128, 1], f32)
        nc.gpsimd.memset(bbar_c[:], 1.0 + EPS * (T + 1.0))
        eye4_f = consts.tile([B_LOC, B_LOC], f32)
        masks.make_identity(nc, eye4_f[:])
        dummy1 = consts.tile([1, 1], f32)
        nc.gpsimd.memset(dummy1[:], 1.0)

        # ---- long-lived accumulators ----
        n2_all = accp.tile([128, NCOL], f32)
        rinv_all = accp.tile([128, NCOL], f32)
        rs_all = accp.tile([128, NCOL], f32)
        prs_all = accp.tile([128, NCOL], f32)
        ac_all = accp.tile([128, NCOL], f32)
        a2_all = accp.tile([128, NCOL], f32)
        et_all = accp.tile([128, NCOL, D], f32)
        neT_all = accp.tile([128, B_LOC * T], bf16)
        svec_sb = accp.tile([64, B_LOC], bf16)
        zpf = accp.tile([128, K, NCOL], f32)
        zpb = accp.tile([128, K, NCOL], bf16)
        wsbc_all = accp.tile([128, 2 * K * B_LOC], f32)

        # ================= DMAs =================
        # Token index mapping t = p*16 + g (partition-major) makes each
        # partition's slice of a batch one contiguous 4KB HBM run -> cheap
        # descriptors + full DMA bandwidth.  The loss is permutation-
        # invariant over tokens, so only the rearrange strings change.
        dummy1o = consts.tile([1, 1], f32)
        # warm the ln table set while DMAs stream (only set we ever use)
        nc.scalar.activation(dummy1o[:], dummy1[:], AF.Ln)
        ticrow = accp.tile([B_LOC, T], f32)
        tpr = accp.tile([128, B_LOC, NBLK], f32)
        emb_r = emb_d.rearrange("b (p g) d -> p b (g d)", p=128)
        et_flat = et_all[:].rearrange("p c d -> p (c d)")
        # batch 0 (critical path): 3-way split across all DMA-capable queues
        c0, c1 = 6 * D, 11 * D
        nc.sync.dma_start(et_flat[:, 0:c0], emb_r[:, 0, 0:c0])
        nc.scalar.dma_start(et_flat[:, c0:c1], emb_r[:, 0, c0:c1])
        nc.gpsimd.dma_start(
            et_flat[:, c1 : NBLK * D], emb_r[:, 0, c1 : NBLK * D]
        )
        nc.gpsimd.dma_start(ticrow[:], tic_d)
        nc.gpsimd.dma_start(
            tpr[:], tic_d.rearrange("b (p g) -> p b g", p=128)
        )
        HB = NBLK * D // 2
        for b in range(1, B_LOC):
            for h, eng in enumerate((nc.sync, nc.scalar)):
                eng.dma_start(
                    et_flat[:, b * NBLK * D + h * HB : b * NBLK * D + (h + 1) * HB],
                    emb_r[:, b, h * HB : (h + 1) * HB],
                )

        # ---- norms over a block range: squares + segmented reduce ---------
        def squares_cols(c0, c1):
            sqb = psq.tile([128, NBLK // 2, D], f32, tag="sqbig")
            sl3 = et_all[:, c0:c1, :]
            nc.vector.tensor_mul(sqb[:, 0 : c1 - c0, :], sl3, sl3)
            nc.vector.reduce_sum(
                n2_all[:, c0:c1], sqb[:, 0 : c1 - c0, :], axis=AX.X
            )

        i32 = mybir.dt.int32
        RSQRT_MAGIC = float(0x5F3759DF)

        def rinv_cols(c0, c1):
            # rinv = 1/sqrt(n2) on DVE: bit-hack seed + 2 Newton iterations
            # (keeps ACT free for Ln and avoids sqrt-table thrash)
            w = c1 - c0
            sl = n2_all[:, c0:c1]
            sh = small.tile([128, NBLK], i32, tag="rsh")
            nc.vector.tensor_scalar(
                out=sh[:, 0:w], in0=sl.bitcast(i32), scalar1=1, scalar2=None,
                op0=OP.logical_shift_right,
            )
            sd = small.tile([128, NBLK], i32, tag="rsd")
            nc.vector.tensor_scalar(
                out=sd[:, 0:w], in0=sh[:, 0:w], scalar1=-1.0,
                scalar2=RSQRT_MAGIC, op0=OP.mult, op1=OP.add,
            )
            y = sd[:, 0:w].bitcast(f32)
            for it in range(2):
                t1 = small.tile([128, NBLK], f32, tag=f"rn{it}a")
                nc.vector.tensor_mul(t1[:, 0:w], y, y)
                t2 = small.tile([128, NBLK], f32, tag=f"rn{it}b")
                nc.vector.tensor_mul(t2[:, 0:w], t1[:, 0:w], sl)
                t3 = small.tile([128, NBLK], f32, tag=f"rn{it}c")
                nc.vector.tensor_scalar(
                    out=t3[:, 0:w], in0=t2[:, 0:w], scalar1=-0.5, scalar2=1.5,
                    op0=OP.mult, op1=OP.add,
                )
                if it == 1:
                    yn = rinv_all[:, c0:c1]
                else:
                    ynt = small.tile([128, NBLK], f32, tag="rny", name="rny")
                    yn = ynt[:, 0:w]
                nc.vector.tensor_mul(yn, y, t3[:, 0:w])
                y = yn

        # ================= Phase T: tic -> z powers =================
        z_all = accp.tile([128, NCOL], f32)

        def phaseT_z():
            mx4 = small.tile([B_LOC, 4], f32, tag="mx4")
            tic3 = ticrow[:].rearrange("b (q s) -> b q s", q=4)
            for q in range(4):
                nc.vector.reduce_max(mx4[:, q : q + 1], tic3[:, q, :],
                                     axis=AX.X)
            mx = small.tile([B_LOC, 1], f32, tag="mx")
            nc.vector.reduce_max(mx[:], mx4[:], axis=AX.X)
            rmx = small.tile([B_LOC, 1], f32, tag="rmx")
            nc.vector.reciprocal(rmx[:], mx[:])
            rrow_ps = pG.tile([1, B_LOC], f32, tag="g")
            nc.tensor.matmul(rrow_ps[:], rmx[:], eye4_f[:])
            rrow_sb = small.tile([1, B_LOC], f32, tag="rrow")
            nc.vector.tensor_scalar(
                out=rrow_sb[:], in0=rrow_ps[:], scalar1=INV_SQRT2,
                scalar2=None, op0=OP.mult,
            )
            rbc_ps = pG.tile([128, B_LOC], f32, tag="g")
            nc.tensor.matmul(rbc_ps[:], ones_row_f[:], rrow_sb[:])
            rbc_sb = small.tile([128, B_LOC], f32, tag="rbc")
            nc.vector.tensor_copy(rbc_sb[:], rbc_ps[:])
            for b in range(B_LOC):
                nc.vector.tensor_scalar(
                    out=z_all[:, b * NBLK : (b + 1) * NBLK],
                    in0=tpr[:, b, :],
                    scalar1=rbc_sb[:, b : b + 1], scalar2=ZC,
                    op0=OP.mult, op1=OP.subtract,
                )
            nc.gpsimd.memset(zpf[:, 0, :], 1.0)
            nc.vector.tensor_copy(zpf[:, 1, :], z_all[:])
            for m in range(2, K):
                nc.vector.tensor_mul(zpf[:, m, :], zpf[:, m - 1, :], z_all[:])
            nc.vector.tensor_copy(zpb[:], zpf[:])

        def phaseT_sws(b):
            s_ps = pG.tile([K, 1], f32, tag="g")
            for j in range(NBLK):
                nc.tensor.matmul(
                    s_ps[:], zpb[:, :, b * NBLK + j], ones_col_b[:],
                    start=(j == 0), stop=(j == NBLK - 1),
                )
            s_sb = small.tile([K, 1], f32, tag="ssb")
            nc.vector.tensor_copy(s_sb[:], s_ps[:])
            ws_ps = pG.tile([1, 2 * K], f32, tag="g")
            nc.tensor.matmul(ws_ps[:], s_sb[:], gam_f[:])
            ws_row = small.tile([1, 2 * K], f32, tag="wsrow")
            nc.vector.tensor_copy(ws_row[:], ws_ps[:])
            wsbc_ps = pG.tile([128, 2 * K], f32, tag="g")
            nc.tensor.matmul(wsbc_ps[:], ones_row_f[:], ws_row[:])
            nc.vector.tensor_copy(
                wsbc_all[:, b * 2 * K : (b + 1) * 2 * K], wsbc_ps[:]
            )

        # batch-0 Phase E as three chunk pipelines matched to its DMA splits
        B0_CHUNKS = [(0, 6), (6, 11), (11, 16)]

        # ---- per-batch Phase E pieces (upfront for b0, interleaved into the
        #      previous batch's j-loop for b1..b3) ----
        ne2_tiles = {}
        pending_tail = []
        pending_rs = []
        svbc_all = accp.tile([128, B_LOC * D], f32)

        def phaseE_block(b, k, on_pe=False):
            c = b * NBLK + k
            ne2 = pne.tile([128, 128], bf16, tag=f"ne{k}")
            src2 = et_all[:, c, :].unsqueeze(1).broadcast_to((128, 2, D))
            ne2v = ne2[:].rearrange("p (two d) -> p two d", two=2)
            nc.vector.tensor_scalar(
                out=ne2v, in0=src2,
                scalar1=rinv_all[:, c : c + 1], scalar2=None, op0=OP.mult,
            )
            dst = neT_all[:, b * T + k * 128 : b * T + (k + 1) * 128]
            if on_pe:
                # preamble path: lower latency than the sync-queue XBAR
                t_ps = pG.tile([128, 128], bf16, tag="g")
                nc.tensor.transpose(t_ps[:], ne2[:], identity[:])
                nc.vector.tensor_copy(dst, t_ps[:])
            else:
                # steady state: DMA XBAR transpose (frees PE + DVE)
                nc.sync.dma_start(dst, ne2[:], transpose=True)
            ne2_tiles[(b, k)] = ne2

        def phaseE_sv_bbar(b):
            sv_ps = pG.tile([64, 1], f32, tag="g")
            for k in range(NBLK):
                nc.tensor.matmul(
                    sv_ps[:], ne2_tiles.pop((b, k))[:, 0:64], ones_col_b[:],
                    start=(k == 0), stop=(k == NBLK - 1),
                )
            nc.vector.tensor_copy(svec_sb[:, b : b + 1], sv_ps[:])
            # broadcast svec across partitions (for DVE rs dot products)
            svr_ps = pG.tile([1, 64], f32, tag="g")
            nc.tensor.matmul(
                svr_ps[:], svec_sb[:, b : b + 1], identity[0:64, 0:64]
            )
            svr_sb = small.tile([1, 64], f32, tag="svrow")
            nc.vector.tensor_copy(svr_sb[:], svr_ps[:])
            svbc_ps = pG.tile([128, 64], f32, tag="g")
            nc.tensor.matmul(svbc_ps[:], ones_row_f[:], svr_sb[:])
            nc.vector.tensor_copy(
                svbc_all[:, b * D : (b + 1) * D], svbc_ps[:]
            )
            # rs dots for this batch can start as soon as svbc exists
            pending_rs.extend(
                (lambda bb, ii: lambda: rs_block(bb, ii))(b, i)
                for i in range(NBLK)
            )

        with tc.high_priority():
            for k0, k1 in B0_CHUNKS:
                squares_cols(k0, k1)
                rinv_cols(k0, k1)
                for k in range(k0, k1):
                    phaseE_block(0, k, on_pe=True)
            phaseE_sv_bbar(0)
        phaseT_z()

        # ---- rs dot for one block on DVE: rs = sum_d (et*rinv)*svec_bc ----
        def rs_block(b, i):
            c = b * NBLK + i
            scr64 = small.tile([128, D], f32, tag="scr64")
            nc.vector.scalar_tensor_tensor(
                out=scr64[:], in0=et_all[:, c, :],
                scalar=rinv_all[:, c : c + 1],
                in1=svbc_all[:, b * D : (b + 1) * D],
                op0=OP.mult, op1=OP.mult,
                accum_out=rs_all[:, c : c + 1],
            )

        # ---- prs/a2 via Horner over a whole batch: P(z)=sum_i w_i z^i ----
        def horner_poly(b, which, dest):
            base = b * 2 * K + which * K
            zb = z_all[:, b * NBLK : (b + 1) * NBLK]
            acc = [small.tile([128, NBLK], f32, tag=f"h{which}{p}",
                              name=f"hacc{which}{p}")
                   for p in range(2)]
            nc.vector.tensor_scalar(
                out=acc[0][:], in0=zb, scalar1=0.0,
                scalar2=wsbc_all[:, base + K - 1 : base + K],
                op0=OP.mult, op1=OP.add,
            )
            cur = 0
            for i in range(K - 2, 0, -1):
                nc.vector.scalar_tensor_tensor(
                    out=acc[1 - cur][:], in0=acc[cur][:],
                    scalar=wsbc_all[:, base + i : base + i + 1], in1=zb,
                    op0=OP.add, op1=OP.mult,
                )
                cur = 1 - cur
            nc.vector.tensor_scalar(
                out=dest, in0=acc[cur][:],
                scalar1=wsbc_all[:, base : base + 1], scalar2=None, op0=OP.add,
            )

        # ================= Main loop per batch =================
        def emit_tail_piece():
            if pending_tail:
                pending_tail.pop(0)()

        def make_tail(b, m_sb):
            wt_sbs = {}

            def wt_half(h):
                wt_ps = pG.tile([128, 8 * K], f32, tag="g")
                for i in range(8 * h, 8 * h + 8):
                    nc.tensor.matmul(
                        wt_ps[:, (i - 8 * h) * K : (i - 8 * h + 1) * K],
                        m_sb[:, i * 128 : (i + 1) * 128], gam_b[:],
                    )
                wt_sb = pmw.tile([128, 8 * K], f32, tag=f"wtsb{h}")
                nc.vector.tensor_copy(wt_sb[:], wt_ps[:])
                wt_sbs[h] = wt_sb

            def dots(h):
                wt_sb = wt_sbs[h]
                for i in range(8 * h, 8 * h + 8):
                    col = b * NBLK + i
                    scr9 = small.tile([128, K], f32, tag="scr9")
                    nc.vector.scalar_tensor_tensor(
                        out=scr9[:], in0=zpf[:, :, col], scalar=1.0,
                        in1=wt_sb[:, (i - 8 * h) * K : (i - 8 * h + 1) * K],
                        op0=OP.mult, op1=OP.mult,
                        accum_out=ac_all[:, col : col + 1],
                    )

            return [lambda: wt_half(0), lambda: dots(0),
                    lambda: wt_half(1), lambda: dots(1)]

        def emit_G_Ln(b, j, lnfs):
            neT = neT_all[:, b * T : (b + 1) * T]
            pT_lo = neT[0:64, j * 128 : (j + 1) * 128]
            pT_hi = neT[64:128, j * 128 : (j + 1) * 128]
            lnf = plnf.tile([128, T], bf16, tag="lnf")
            for half in range(2):
                g_ps = pG.tile([128, 1024], f32, tag="g")
                nc.tensor.matmul(
                    g_ps[:, 0:512], pT_lo,
                    neT[0:64, (2 * half) * 512 : (2 * half + 1) * 512],
                )
                nc.tensor.matmul(
                    g_ps[:, 512:1024], pT_hi,
                    neT[64:128, (2 * half + 1) * 512 : (2 * half + 2) * 512],
                )
                nc.scalar.activation(
                    lnf[:, half * 1024 : (half + 1) * 1024], g_ps[:],
                    AF.Ln, bias=bbar_c[:], scale=1.0,
                )
            lnfs[j] = lnf

        def emit_M(b, jm, m_ps, lnfs):
            vj = zpb[:, :, b * NBLK + jm]
            for cch in range(4):
                nc.tensor.matmul(
                    m_ps[:, cch * 512 : (cch + 1) * 512], vj,
                    lnfs[jm][:, cch * 512 : (cch + 1) * 512],
                    start=(jm == 0), stop=(jm == NBLK - 1),
                )

        def finish_batch(b, m_ps, lnfs):
            # last moment matmul + PSUM->SBUF copy of M, queue the tail
            emit_M(b, NBLK - 1, m_ps, lnfs)
            m_sb = pmw.tile([K, T], bf16, tag="msb")
            for cch in range(4):
                nc.vector.tensor_copy(
                    m_sb[:, cch * 512 : (cch + 1) * 512],
                    m_ps[:, cch * 512 : (cch + 1) * 512],
                )
            pending_tail.extend(make_tail(b, m_sb))

        rows4 = accp.tile([128, B_LOC], f32)

        def finals_batch(b):
            sl = slice(b * NBLK, (b + 1) * NBLK)
            lnp = small.tile([128, NBLK], f32, tag="flnp")
            nc.scalar.activation(lnp[:], prs_all[:, sl], AF.Ln)
            lnr = small.tile([128, NBLK], f32, tag="flnr")
            nc.scalar.activation(
                lnr[:], rs_all[:, sl], AF.Ln, bias=bias_T[:], scale=1.0
            )
            rpr = small.tile([128, NBLK], f32, tag="frpr")
            nc.vector.reciprocal(rpr[:], prs_all[:, sl])
            cc = small.tile([128, NBLK], f32, tag="fcc")
            nc.vector.tensor_sub(cc[:], lnr[:], lnp[:])
            acs = small.tile([128, NBLK], f32, tag="facs")
            nc.vector.tensor_add(acs[:], ac_all[:, sl], a2_all[:, sl])
            pq = small.tile([128, NBLK], f32, tag="fpq")
            nc.vector.tensor_mul(pq[:], acs[:], rpr[:])
            rows = small.tile([128, NBLK], f32, tag="frows")
            nc.vector.tensor_sub(rows[:], cc[:], pq[:])
            nc.vector.reduce_sum(rows4[:, b : b + 1], rows[:], axis=AX.X)

        carry = None  # (b, m_ps, lnfs) with M(b, 15) still unemitted
        for b in range(B_LOC):
            m_ps = None
            lnfs = [None] * NBLK
            for j in range(NBLK):
                emit_G_Ln(b, j, lnfs)
                if j == 0 and carry is not None:
                    finish_batch(*carry)
                    carry = None
                if j >= 1:
                    if j == 1:
                        m_ps = pM.tile([K, T], f32, tag="m")
                    emit_M(b, j - 1, m_ps, lnfs)
                # --- interleaved fill-in work ---
                if b == 0 and j == 2:
                    phaseT_sws(0)
                if j == 5:
                    horner_poly(b, 0, prs_all[:, b * NBLK : (b + 1) * NBLK])
                elif j == 6:
                    horner_poly(b, 1, a2_all[:, b * NBLK : (b + 1) * NBLK])
                elif j == 9 and b >= 1:
                    finals_batch(b - 1)
                for _ in range(2):
                    if pending_rs and j >= 1:
                        pending_rs.pop(0)()
                if b + 1 < B_LOC:
                    if j in (1, 2):
                        h = j - 1
                        c0 = (b + 1) * NBLK + h * (NBLK // 2)
                        squares_cols(c0, c0 + NBLK // 2)
                        rinv_cols(c0, c0 + NBLK // 2)
                    elif 3 <= j <= 10:
                        phaseE_block(b + 1, 2 * (j - 3))
                        phaseE_block(b + 1, 2 * (j - 3) + 1)
                    if j == 12:
                        phaseT_sws(b + 1)
                    if j == 11:
                        phaseE_sv_bbar(b + 1)
                if 1 <= j <= 4:
                    emit_tail_piece()
            carry = (b, m_ps, lnfs)

        finish_batch(*carry)
        while pending_rs:
            pending_rs.pop(0)()
        while pending_tail:
            emit_tail_piece()
        finals_batch(B_LOC - 1)

        # ================= finals =================
        tot = accp.tile([128, 1], f32)
        nc.vector.reduce_sum(tot[:], rows4[:], axis=AX.X)
        nc.sync.dma_start(out_d[:], tot[:])

    nc.compile()
    return nc


def _get_program():
    global _PROGRAM
    if _PROGRAM is None:
        _PROGRAM = _build_program()
    return _PROGRAM


def _install_ntff_hook():
    """Register the axon NTFF profiling hook (test/profiling path only)."""
    import sys
    import types

    if "antenv.axon_hooks" not in sys.modules:
        import antenv

        mod = types.ModuleType("antenv.axon_hooks")
        state = {"hook": None}
        mod.set_axon_ntff_profile_hook = lambda h: state.__setitem__("hook", h)
        mod.get_axon_ntff_profile_hook = lambda: state["hook"]
        sys.modules["antenv.axon_hooks"] = mod
        antenv.axon_hooks = mod
        from trn_agent_boot.trn_boot import _ntff_profile_via_ctypes

        mod.set_axon_ntff_profile_hook(
            _ntff_profile_via_ctypes("/opt/axon/libaxon_pjrt.so")
        )
    from concourse import bass_utils

    bass_utils.upload_artifacts = lambda tmpdir: tmpdir


def kernel(embedding, tic, _trace=False):
    embedding = np.ascontiguousarray(embedding, dtype=np.float32)
    tic = np.ascontiguousarray(tic, dtype=np.float32)
    assert embedding.shape == (B, T, D) and tic.shape == (B, T)

    from concourse.bass_utils import run_bass_kernel_spmd

    if _trace:
        _install_ntff_hook()
    try:
        import ctypes
        import jax
        jax.devices()
        ctypes.CDLL("/opt/axon/libaxon_pjrt.so").axon_reset()
    except Exception:
        pass
    nc = _get_program()
    in_maps = [
        {
            "embedding": embedding[i * B_LOC : (i + 1) * B_LOC],
            "tic": tic[i * B_LOC : (i + 1) * B_LOC],
            "gams": GAMS,
        }
        for i in range(N_CORES)
    ]
    res = run_bass_kernel_spmd(nc, in_maps, list(range(N_CORES)), trace=_trace)
    total = sum(float(r["out"].sum()) for r in res.results)
    loss = np.array(total / (B * T), dtype=np.float32)
    if _trace:
        return loss, res
    return loss


# revision 86
# speedup vs baseline: 1.0189x; 1.0189x over previous
"""Trainium2 Bass kernel v3: contrastive KL loss via moment expansion.

Per batch b (anchors p, targets t, T=2048):
    tw = pos/prs, pos = exp(-d2), d2 = (z_p - z_t)^2, z = tic_n/sqrt(2) - ZC
    iw = sc/rsum, sc = G+1 (cosine sim + 1), rsum = rs + T
    row_kl = ln(rs+T) - ln(prs) - (A + C)/prs
      prs = sum_t pos,  A = sum_t pos*d2,  C = sum_t pos*lnf,
      lnf = ln(G + bbar),  bbar = 1 + eps*(T + mean_p rs)   [symmetric bias]
      mean_p rs = |svec|^2 / T with svec = sum_t ne_t  (exact identity)

Moment restructure (validated ~1e-3 vs reference, tol 2e-2):
    pos ~ P(d2) deg-3 poly  ->  pos = sum_{i,j} Gam[i,j] z_p^i z_t^j  (rank 9)
    pos*d2 ~ H(d2) deg-4 poly (GamH)
    M[m,p] = sum_t z_t^m lnf[p,t] = sum_j V_j^T @ lnf_j   (PE, uses lnf symmetry)
    S[m]   = sum_t z_t^m
    Wt_blk = M_blk^T @ GamP^T  (PE, 9-col matmuls; fuses W matmul + transpose)
    AC[p]  = sum_i z_p^i Wt[p,i]   (DVE dot w/ accum)
    prs[p] = sum_i z_p^i (GamP @ S)_i,  A2[p] = sum_i z_p^i (GamH @ S)_i

v3 scheduling (vs v2 baseline at 315us):
    - 8 wide DMAs across 4 engine queues instead of 64 serialized on Sync
    - n2 squares on DVE (not ACT); dummy Sqrt at t=0 preloads sqrt table
    - bbar from |svec|^2: first Ln of each batch has no rs dependency
    - rs matvecs after each j-loop; prs/a2 dots + batch tails + next-batch
      Phase E interleaved into the j-loop so ACT (the ~147us Ln floor) never
      stalls and PE stays HAM-warm.
"""

import os
import numpy as np

os.environ.setdefault("MYCRO_LOCAL_CACHE", "1")

B, T, D = 32, 2048, 64
N_CORES = 8
B_LOC = B // N_CORES
NBLK = T // 128
EPS = 1e-6
ZC = 0.35
INV_SQRT2 = 0.7071067811865476
NP_DEG, NH_DEG = 3, 4
K = 2 * max(NP_DEG, NH_DEG) + 1      # 9 monomials z^0..z^8
NCOL = B_LOC * NBLK                  # 64 accumulator columns per core

_PROGRAM = None


def _fit_gammas():
    from math import comb
    xs = np.linspace(0.0, 0.5, 4001)
    cP = np.polyfit(xs, np.exp(-xs), NP_DEG)[::-1]
    cH = np.polyfit(xs, xs * np.exp(-xs), NH_DEG)[::-1]

    def gamma_of(c):
        g = np.zeros((K, K))
        for k_, ck in enumerate(c):
            for j in range(2 * k_ + 1):
                g[2 * k_ - j, j] += ck * comb(2 * k_, j) * (-1) ** j
        return g

    gp, gh = gamma_of(cP), gamma_of(cH)
    # device layout: gams[j, i] = Gam[i, j]  (lhsT for W = Gam @ M)
    return np.concatenate([gp.T, gh.T], axis=1).astype(np.float32)  # [K, 2K]


GAMS = _fit_gammas()


def _build_program():
    from contextlib import ExitStack
    import concourse.bass as bass
    import concourse.tile as tile
    from concourse import bacc, mybir, masks

    f32 = mybir.dt.float32
    bf16 = mybir.dt.bfloat16
    AF = mybir.ActivationFunctionType
    OP = mybir.AluOpType
    AX = mybir.AxisListType

    nc = bacc.Bacc(
        "TRN2", target_bir_lowering=False, debug=False, num_devices=N_CORES
    )
    emb_d = nc.dram_tensor("embedding", [B_LOC, T, D], f32, kind="ExternalInput").ap()
    tic_d = nc.dram_tensor("tic", [B_LOC, T], f32, kind="ExternalInput").ap()
    gam_d = nc.dram_tensor("gams", [K, 2 * K], f32, kind="ExternalInput").ap()
    out_d = nc.dram_tensor("out", [128, 1], f32, kind="ExternalOutput").ap()

    with tile.TileContext(nc) as tc, ExitStack() as ctx:
        consts = ctx.enter_context(tc.tile_pool(name="consts", bufs=1))
        accp = ctx.enter_context(tc.tile_pool(name="acc", bufs=1))
        small = ctx.enter_context(tc.tile_pool(name="small", bufs=12))
        psq = ctx.enter_context(tc.tile_pool(name="psq", bufs=3))
        pne = ctx.enter_context(tc.tile_pool(name="pne", bufs=18))
        plnf = ctx.enter_context(tc.tile_pool(name="plnf", bufs=6))
        pmw = ctx.enter_context(tc.tile_pool(name="pmw", bufs=4))
        # PSUM: pG 2x[128,1024]f32 = 4 banks, pM 1x[9,2048]f32 = 4 banks
        pG = ctx.enter_context(
            tc.tile_pool(name="pG", bufs=2, space=bass.MemorySpace.PSUM)
        )
        pM = ctx.enter_context(
            tc.tile_pool(name="pM", bufs=1, space=bass.MemorySpace.PSUM)
        )

        # ---- constants ----
        identity = consts.tile([128, 128], bf16)
        masks.make_identity(nc, identity[:])
        ones_row_f = consts.tile([1, 128], f32)
        nc.gpsimd.memset(ones_row_f[:], 1.0)
        ones_col_b = consts.tile([128, 1], bf16)
        nc.gpsimd.memset(ones_col_b[:], 1.0)
        gam_f = consts.tile([K, 2 * K], f32)
        nc.sync.dma_start(gam_f[:], gam_d)
        gam_b = consts.tile([K, K], bf16)
        nc.vector.tensor_copy(gam_b[:], gam_f[:, 0:K])
        bias_T = consts.tile([128, 1], f32)
        nc.gpsimd.memset(bias_T[:], float(T))
        # bbar = 1 + eps*(T + mean_p rs); mean_p rs ~ 1 +- 0.5 and eps=1e-6,
        # so a constant bias is accurate to ~1e-6 absolute in lnf
        bbar_c = consts.tile([128, 1], f32)
        nc.gpsimd.memset(bbar_c[:], 1.0 + EPS * (T + 1.0))
        eye4_f = consts.tile([B_LOC, B_LOC], f32)
        masks.make_identity(nc, eye4_f[:])
        dummy1 = consts.tile([1, 1], f32)
        nc.gpsimd.memset(dummy1[:], 1.0)

        # ---- long-lived accumulators ----
        n2_all = accp.tile([128, NCOL], f32)
        rinv_all = accp.tile([128, NCOL], f32)
        rs_all = accp.tile([128, NCOL], f32)
        prs_all = accp.tile([128, NCOL], f32)
        ac_all = accp.tile([128, NCOL], f32)
        a2_all = accp.tile([128, NCOL], f32)
        et_all = accp.tile([128, NCOL, D], f32)
        neT_all = accp.tile([128, B_LOC * T], bf16)
        svec_sb = accp.tile([64, B_LOC], bf16)
        zpf = accp.tile([128, K, NCOL], f32)
        zpb = accp.tile([128, K, NCOL], bf16)
        wsbc_all = accp.tile([128, 2 * K * B_LOC], f32)

        # ================= DMAs =================
        # Token index mapping t = p*16 + g (partition-major) makes each
        # partition's slice of a batch one contiguous 4KB HBM run -> cheap
        # descriptors + full DMA bandwidth.  The loss is permutation-
        # invariant over tokens, so only the rearrange strings change.
        dummy1o = consts.tile([1, 1], f32)
        # warm the ln table set while DMAs stream (only set we ever use)
        nc.scalar.activation(dummy1o[:], dummy1[:], AF.Ln)
        ticrow = accp.tile([B_LOC, T], f32)
        tpr = accp.tile([128, B_LOC, NBLK], f32)
        emb_r = emb_d.rearrange("b (p g) d -> p b (g d)", p=128)
        et_flat = et_all[:].rearrange("p c d -> p (c d)")
        # batch 0 (critical path): 3-way split across all DMA-capable queues
        c0, c1 = 6 * D, 11 * D
        nc.sync.dma_start(et_flat[:, 0:c0], emb_r[:, 0, 0:c0])
        nc.scalar.dma_start(et_flat[:, c0:c1], emb_r[:, 0, c0:c1])
        nc.gpsimd.dma_start(
            et_flat[:, c1 : NBLK * D], emb_r[:, 0, c1 : NBLK * D]
        )
        nc.gpsimd.dma_start(ticrow[:], tic_d)
        nc.gpsimd.dma_start(
            tpr[:], tic_d.rearrange("b (p g) -> p b g", p=128)
        )
        HB = NBLK * D // 2
        for b in range(1, B_LOC):
            for h, eng in enumerate((nc.sync, nc.scalar)):
                eng.dma_start(
                    et_flat[:, b * NBLK * D + h * HB : b * NBLK * D + (h + 1) * HB],
                    emb_r[:, b, h * HB : (h + 1) * HB],
                )

        # ---- norms over a block range: squares + segmented reduce ---------
        def squares_cols(c0, c1):
            sqb = psq.tile([128, NBLK // 2, D], f32, tag="sqbig")
            sl3 = et_all[:, c0:c1, :]
            nc.vector.tensor_mul(sqb[:, 0 : c1 - c0, :], sl3, sl3)
            nc.vector.reduce_sum(
                n2_all[:, c0:c1], sqb[:, 0 : c1 - c0, :], axis=AX.X
            )

        i32 = mybir.dt.int32
        RSQRT_MAGIC = float(0x5F3759DF)

        def rinv_cols(c0, c1):
            # rinv = 1/sqrt(n2) on DVE: bit-hack seed + 2 Newton iterations
            # (keeps ACT free for Ln and avoids sqrt-table thrash)
            w = c1 - c0
            sl = n2_all[:, c0:c1]
            sh = small.tile([128, NBLK], i32, tag="rsh")
            nc.vector.tensor_scalar(
                out=sh[:, 0:w], in0=sl.bitcast(i32), scalar1=1, scalar2=None,
                op0=OP.logical_shift_right,
            )
            sd = small.tile([128, NBLK], i32, tag="rsd")
            nc.vector.tensor_scalar(
                out=sd[:, 0:w], in0=sh[:, 0:w], scalar1=-1.0,
                scalar2=RSQRT_MAGIC, op0=OP.mult, op1=OP.add,
            )
            y = sd[:, 0:w].bitcast(f32)
            for it in range(2):
                t1 = small.tile([128, NBLK], f32, tag=f"rn{it}a")
                nc.vector.tensor_mul(t1[:, 0:w], y, y)
                t2 = small.tile([128, NBLK], f32, tag=f"rn{it}b")
                nc.vector.tensor_mul(t2[:, 0:w], t1[:, 0:w], sl)
                t3 = small.tile([128, NBLK], f32, tag=f"rn{it}c")
                nc.vector.tensor_scalar(
                    out=t3[:, 0:w], in0=t2[:, 0:w], scalar1=-0.5, scalar2=1.5,
                    op0=OP.mult, op1=OP.add,
                )
                if it == 1:
                    yn = rinv_all[:, c0:c1]
                else:
                    ynt = small.tile([128, NBLK], f32, tag="rny", name="rny")
                    yn = ynt[:, 0:w]
                nc.vector.tensor_mul(yn, y, t3[:, 0:w])
                y = yn

        # ================= Phase T: tic -> z powers =================
        z_all = accp.tile([128, NCOL], f32)

        def phaseT_z():
            mx4 = small.tile([B_LOC, 4], f32, tag="mx4")
            tic3 = ticrow[:].rearrange("b (q s) -> b q s", q=4)
            for q in range(4):
                nc.vector.reduce_max(mx4[:, q : q + 1], tic3[:, q, :],
                                     axis=AX.X)
            mx = small.tile([B_LOC, 1], f32, tag="mx")
            nc.vector.reduce_max(mx[:], mx4[:], axis=AX.X)
            rmx = small.tile([B_LOC, 1], f32, tag="rmx")
            nc.vector.reciprocal(rmx[:], mx[:])
            rrow_ps = pG.tile([1, B_LOC], f32, tag="g")
            nc.tensor.matmul(rrow_ps[:], rmx[:], eye4_f[:])
            rrow_sb = small.tile([1, B_LOC], f32, tag="rrow")
            nc.vector.tensor_scalar(
                out=rrow_sb[:], in0=rrow_ps[:], scalar1=INV_SQRT2,
                scalar2=None, op0=OP.mult,
            )
            rbc_ps = pG.tile([128, B_LOC], f32, tag="g")
            nc.tensor.matmul(rbc_ps[:], ones_row_f[:], rrow_sb[:])
            rbc_sb = small.tile([128, B_LOC], f32, tag="rbc")
            nc.vector.tensor_copy(rbc_sb[:], rbc_ps[:])
            for b in range(B_LOC):
                nc.vector.tensor_scalar(
                    out=z_all[:, b * NBLK : (b + 1) * NBLK],
                    in0=tpr[:, b, :],
                    scalar1=rbc_sb[:, b : b + 1], scalar2=ZC,
                    op0=OP.mult, op1=OP.subtract,
                )
            nc.gpsimd.memset(zpf[:, 0, :], 1.0)
            nc.vector.tensor_copy(zpf[:, 1, :], z_all[:])
            for m in range(2, K):
                nc.vector.tensor_mul(zpf[:, m, :], zpf[:, m - 1, :], z_all[:])
            nc.vector.tensor_copy(zpb[:], zpf[:])

        def phaseT_sws(b):
            s_ps = pG.tile([K, 1], f32, tag="g")
            for j in range(NBLK):
                nc.tensor.matmul(
                    s_ps[:], zpb[:, :, b * NBLK + j], ones_col_b[:],
                    start=(j == 0), stop=(j == NBLK - 1),
                )
            s_sb = small.tile([K, 1], f32, tag="ssb")
            nc.vector.tensor_copy(s_sb[:], s_ps[:])
            ws_ps = pG.tile([1, 2 * K], f32, tag="g")
            nc.tensor.matmul(ws_ps[:], s_sb[:], gam_f[:])
            ws_row = small.tile([1, 2 * K], f32, tag="wsrow")
            nc.vector.tensor_copy(ws_row[:], ws_ps[:])
            wsbc_ps = pG.tile([128, 2 * K], f32, tag="g")
            nc.tensor.matmul(wsbc_ps[:], ones_row_f[:], ws_row[:])
            nc.vector.tensor_copy(
                wsbc_all[:, b * 2 * K : (b + 1) * 2 * K], wsbc_ps[:]
            )

        # batch-0 Phase E as three chunk pipelines matched to its DMA splits
        B0_CHUNKS = [(0, 6), (6, 11), (11, 16)]

        # ---- per-batch Phase E pieces (upfront for b0, interleaved into the
        #      previous batch's j-loop for b1..b3) ----
        ne2_tiles = {}
        pending_tail = []
        pending_rs = []
        svbc_all = accp.tile([128, B_LOC * D], f32)

        def phaseE_block(b, k, on_pe=False):
            c = b * NBLK + k
            ne2 = pne.tile([128, 128], bf16, tag=f"ne{k}")
            src2 = et_all[:, c, :].unsqueeze(1).broadcast_to((128, 2, D))
            ne2v = ne2[:].rearrange("p (two d) -> p two d", two=2)
            nc.vector.tensor_scalar(
                out=ne2v, in0=src2,
                scalar1=rinv_all[:, c : c + 1], scalar2=None, op0=OP.mult,
            )
            dst = neT_all[:, b * T + k * 128 : b * T + (k + 1) * 128]
            if on_pe:
                # preamble path: lower latency than the sync-queue XBAR
                t_ps = pG.tile([128, 128], bf16, tag="g")
                nc.tensor.transpose(t_ps[:], ne2[:], identity[:])
                nc.vector.tensor_copy(dst, t_ps[:])
            else:
                # steady state: DMA XBAR transpose (frees PE + DVE)
                nc.sync.dma_start(dst, ne2[:], transpose=True)
            ne2_tiles[(b, k)] = ne2

        def phaseE_sv_bbar(b):
            sv_ps = pG.tile([64, 1], f32, tag="g")
            for k in range(NBLK):
                nc.tensor.matmul(
                    sv_ps[:], ne2_tiles.pop((b, k))[:, 0:64], ones_col_b[:],
                    start=(k == 0), stop=(k == NBLK - 1),
                )
            nc.vector.tensor_copy(svec_sb[:, b : b + 1], sv_ps[:])
            # broadcast svec across partitions (for DVE rs dot products)
            svr_ps = pG.tile([1, 64], f32, tag="g")
            nc.tensor.matmul(
                svr_ps[:], svec_sb[:, b : b + 1], identity[0:64, 0:64]
            )
            svr_sb = small.tile([1, 64], f32, tag="svrow")
            nc.vector.tensor_copy(svr_sb[:], svr_ps[:])
            svbc_ps = pG.tile([128, 64], f32, tag="g")
            nc.tensor.matmul(svbc_ps[:], ones_row_f[:], svr_sb[:])
            nc.vector.tensor_copy(
                svbc_all[:, b * D : (b + 1) * D], svbc_ps[:]
            )
            # rs dots for this batch can start as soon as svbc exists
            pending_rs.extend(
                (lambda bb, ii: lambda: rs_block(bb, ii))(b, i)
                for i in range(NBLK)
            )

        with tc.high_priority():
            for k0, k1 in B0_CHUNKS:
                squares_cols(k0, k1)
                rinv_cols(k0, k1)
                for k in range(k0, k1):
                    phaseE_block(0, k, on_pe=True)
            phaseE_sv_bbar(0)
        phaseT_z()

        # ---- rs dot for one block on DVE: rs = sum_d (et*rinv)*svec_bc ----
        def rs_block(b, i):
            c = b * NBLK + i
            scr64 = small.tile([128, D], f32, tag="scr64")
            nc.vector.scalar_tensor_tensor(
                out=scr64[:], in0=et_all[:, c, :],
                scalar=rinv_all[:, c : c + 1],
                in1=svbc_all[:, b * D : (b + 1) * D],
                op0=OP.mult, op1=OP.mult,
                accum_out=rs_all[:, c : c + 1],
            )

        # ---- prs/a2 via Horner over a whole batch: P(z)=sum_i w_i z^i ----
        def horner_poly(b, which, dest):
            base = b * 2 * K + which * K
            zb = z_all[:, b * NBLK : (b + 1) * NBLK]
            acc = [small.tile([128, NBLK], f32, tag=f"h{which}{p}",
                              name=f"hacc{which}{p}")
                   for p in range(2)]
            nc.vector.tensor_scalar(
                out=acc[0][:], in0=zb, scalar1=0.0,
                scalar2=wsbc_all[:, base + K - 1 : base + K],
                op0=OP.mult, op1=OP.add,
            )
            cur = 0
            for i in range(K - 2, 0, -1):
                nc.vector.scalar_tensor_tensor(
                    out=acc[1 - cur][:], in0=acc[cur][:],
                    scalar=wsbc_all[:, base + i : base + i + 1], in1=zb,
                    op0=OP.add, op1=OP.mult,
                )
                cur = 1 - cur
            nc.vector.tensor_scalar(
                out=dest, in0=acc[cur][:],
                scalar1=wsbc_all[:, base : base + 1], scalar2=None, op0=OP.add,
            )

        # ================= Main loop per batch =================
        def emit_tail_piece():
            if pending_tail:
                pending_tail.pop(0)()

        def make_tail(b, m_sb):
            wt_sbs = {}

            def wt_half(h):
                wt_ps = pG.tile([128, 8 * K], f32, tag="g")
                for i in range(8 * h, 8 * h + 8):
                    nc.tensor.matmul(
                        wt_ps[:, (i - 8 * h) * K : (i - 8 * h + 1) * K],
                        m_sb[:, i * 128 : (i + 1) * 128], gam_b[:],
                    )
                wt_sb = pmw.tile([128, 8 * K], f32, tag=f"wtsb{h}")
                nc.vector.tensor_copy(wt_sb[:], wt_ps[:])
                wt_sbs[h] = wt_sb

            def dots(h):
                wt_sb = wt_sbs[h]
                for i in range(8 * h, 8 * h + 8):
                    col = b * NBLK + i
                    scr9 = small.tile([128, K], f32, tag="scr9")
                    nc.vector.scalar_tensor_tensor(
                        out=scr9[:], in0=zpf[:, :, col], scalar=1.0,
                        in1=wt_sb[:, (i - 8 * h) * K : (i - 8 * h + 1) * K],
                        op0=OP.mult, op1=OP.mult,
                        accum_out=ac_all[:, col : col + 1],
                    )

            return [lambda: wt_half(0), lambda: dots(0),
                    lambda: wt_half(1), lambda: dots(1)]

        def emit_G_Ln(b, j, lnfs):
            neT = neT_all[:, b * T : (b + 1) * T]
            pT_lo = neT[0:64, j * 128 : (j + 1) * 128]
            pT_hi = neT[64:128, j * 128 : (j + 1) * 128]
            lnf = plnf.tile([128, T], bf16, tag="lnf")
            for half in range(2):
                g_ps = pG.tile([128, 1024], f32, tag="g")
                nc.tensor.matmul(
                    g_ps[:, 0:512], pT_lo,
                    neT[0:64, (2 * half) * 512 : (2 * half + 1) * 512],
                )
                nc.tensor.matmul(
                    g_ps[:, 512:1024], pT_hi,
                    neT[64:128, (2 * half + 1) * 512 : (2 * half + 2) * 512],
                )
                nc.scalar.activation(
                    lnf[:, half * 1024 : (half + 1) * 1024], g_ps[:],
                    AF.Ln, bias=bbar_c[:], scale=1.0,
                )
            lnfs[j] = lnf

        def emit_M(b, jm, m_ps, lnfs):
            vj = zpb[:, :, b * NBLK + jm]
            for cch in range(4):
                nc.tensor.matmul(
                    m_ps[0:K, cch * 512 : (cch + 1) * 512], vj,
                    lnfs[jm][:, cch * 512 : (cch + 1) * 512],
                    start=(jm == 0), stop=(jm == NBLK - 1),
                )

        def finish_batch(b, m_ps, lnfs):
            # last moment matmul + PSUM->SBUF copy of M, queue the tail
            emit_M(b, NBLK - 1, m_ps, lnfs)
            m_sb = pmw.tile([K, T], bf16, tag="msb")
            for cch in range(4):
                nc.vector.tensor_copy(
                    m_sb[:, cch * 512 : (cch + 1) * 512],
                    m_ps[0:K, cch * 512 : (cch + 1) * 512],
                )
            pending_tail.extend(make_tail(b, m_sb))

        rows4 = accp.tile([128, B_LOC], f32)

        def finals_batch(b):
            sl = slice(b * NBLK, (b + 1) * NBLK)
            lnp = small.tile([128, NBLK], f32, tag="flnp")
            nc.scalar.activation(lnp[:], prs_all[:, sl], AF.Ln)
            lnr = small.tile([128, NBLK], f32, tag="flnr")
            nc.scalar.activation(
                lnr[:], rs_all[:, sl], AF.Ln, bias=bias_T[:], scale=1.0
            )
            rpr = small.tile([128, NBLK], f32, tag="frpr")
            nc.vector.reciprocal(rpr[:], prs_all[:, sl])
            cc = small.tile([128, NBLK], f32, tag="fcc")
            nc.vector.tensor_sub(cc[:], lnr[:], lnp[:])
            acs = small.tile([128, NBLK], f32, tag="facs")
            nc.vector.tensor_add(acs[:], ac_all[:, sl], a2_all[:, sl])
            pq = small.tile([128, NBLK], f32, tag="fpq")
            nc.vector.tensor_mul(pq[:], acs[:], rpr[:])
            rows = small.tile([128, NBLK], f32, tag="frows")
            nc.vector.tensor_sub(rows[:], cc[:], pq[:])
            nc.vector.reduce_sum(rows4[:, b : b + 1], rows[:], axis=AX.X)

        carry = None  # (b, m_ps, lnfs) with M(b, 15) still unemitted
        for b in range(B_LOC):
            m_ps = None
            lnfs = [None] * NBLK
            for j in range(NBLK):
                emit_G_Ln(b, j, lnfs)
                if j == 0 and carry is not None:
                    finish_batch(*carry)
                    carry = None
                if j >= 1:
                    if j == 1:
                        m_ps = pM.tile([128, T], f32, tag="m")
                    emit_M(b, j - 1, m_ps, lnfs)
                    # HAM keeper: lowest-priority 1-col matmuls execute only
                    # when PE is otherwise idle, so the clock gate stays 8/8
                    with tc.high_priority(offset=-(1 << 20)):
                        for _ in range(4):
                            nc.tensor.matmul(
                                m_ps[64:65, 0:1], ones_col_b[0:1, :],
                                ones_col_b[0:1, :], skip_group_check=True,
                            )
                # --- interleaved fill-in work ---
                if b == 0 and j == 2:
                    phaseT_sws(0)
                if j == 5:
                    horner_poly(b, 0, prs_all[:, b * NBLK : (b + 1) * NBLK])
                elif j == 6:
                    horner_poly(b, 1, a2_all[:, b * NBLK : (b + 1) * NBLK])
                elif j == 9 and b >= 1:
                    finals_batch(b - 1)
                for _ in range(2):
                    if pending_rs and j >= 1:
                        pending_rs.pop(0)()
                if b + 1 < B_LOC:
                    if j in (1, 2):
                        h = j - 1
                        c0 = (b + 1) * NBLK + h * (NBLK // 2)
                        squares_cols(c0, c0 + NBLK // 2)
                        rinv_cols(c0, c0 + NBLK // 2)
                    elif 3 <= j <= 10:
                        phaseE_block(b + 1, 2 * (j - 3))
                        phaseE_block(b + 1, 2 * (j - 3) + 1)
                    if j == 12:
                        phaseT_sws(b + 1)
                    if j == 11:
                        phaseE_sv_bbar(b + 1)
                if 1 <= j <= 4:
                    emit_tail_piece()
            carry = (b, m_ps, lnfs)

        finish_batch(*carry)
        while pending_rs:
            pending_rs.pop(0)()
        while pending_tail:
            emit_tail_piece()
        finals_batch(B_LOC - 1)

        # ================= finals =================
        tot = accp.tile([128, 1], f32)
        nc.vector.reduce_sum(tot[:], rows4[:], axis=AX.X)
        nc.sync.dma_start(out_d[:], tot[:])

    nc.compile()
    return nc


def _get_program():
    global _PROGRAM
    if _PROGRAM is None:
        _PROGRAM = _build_program()
    return _PROGRAM


def _install_ntff_hook():
    """Register the axon NTFF profiling hook (test/profiling path only)."""
    import sys
    import types

    if "antenv.axon_hooks" not in sys.modules:
        import antenv

        mod = types.ModuleType("antenv.axon_hooks")
        state = {"hook": None}
        mod.set_axon_ntff_profile_hook = lambda h: state.__setitem__("hook", h)
        mod.get_axon_ntff_profile_hook = lambda: state["hook"]
        sys.modules["antenv.axon_hooks"] = mod
        antenv.axon_hooks = mod
        from trn_agent_boot.trn_boot import _ntff_profile_via_ctypes

        mod.set_axon_ntff_profile_hook(
            _ntff_profile_via_ctypes("/opt/axon/libaxon_pjrt.so")
        )
    from concourse import bass_utils

    bass_utils.upload_artifacts = lambda tmpdir: tmpdir


def kernel(embedding, tic, _trace=False):
    embedding = np.ascontiguousarray(embedding, dtype=np.float32)
    tic = np.ascontiguousarray(tic, dtype=np.float32)
    assert embedding.shape == (B, T, D) and tic.shape == (B, T)

    from concourse.bass_utils import run_bass_kernel_spmd

    if _trace:
        _install_ntff_hook()
    try:
        import ctypes
        import jax
        jax.devices()
        ctypes.CDLL("/opt/axon/libaxon_pjrt.so").axon_reset()
    except Exception:
        pass
    nc = _get_program()
    in_maps = [
        {
            "embedding": embedding[i * B_LOC : (i + 1) * B_LOC],
            "tic": tic[i * B_LOC : (i + 1) * B_LOC],
            "gams": GAMS,
        }
        for i in range(N_CORES)
    ]
    res = run_bass_kernel_spmd(nc, in_maps, list(range(N_CORES)), trace=_trace)
    total = sum(float(r["out"].sum()) for r in res.results)
    loss = np.array(total / (B * T), dtype=np.float32)
    if _trace:
        return loss, res
    return loss
